# revision 22
# baseline (speedup 1.0000x reference)
"""Trainium2 Bass kernel for DWPEDecomposition.

Problem: x [128, 65536] f32.  For each batch row: full 8-level Haar (db1)
wavelet-packet tree -> [256 nodes, 256 coeffs] in frequency order, per-node
Shannon entropy of the normalized squared coefficients, and a keep mask
(entropy > 0.1) that zeroes pruned nodes' coefficients.

Key insight: the 8-level Haar packet cascade on a 65536-sample row is exactly
a 256x256 Walsh-Hadamard-style matrix W' (entries +-(1/sqrt2)^8, with the
frequency-order node permutation folded in) applied independently to each of
the 256 contiguous 256-sample blocks:

    coeffs[b, n, k] = sum_j W'[n, j] * x[b, 256*k + j]

which maps onto the TensorEngine as (PE-transpose + matmul) per tile.

Matmul precision scheme ("bf16_hilo"): all W' entries share one magnitude, so
bf16(W') = +-0.0625 = W'/rho exactly, with rho a single uniform scalar.  The
transposed x is split into bf16 hi + bf16 lo (x ~ hi+lo to ~2^-17); the four
accumulating bf16 matmuls per output bank compute c_raw = bf16(W') @ (hi+lo)
in fp32 PSUM, and the uniform rho is re-applied exactly downstream (Square's
scale immediate, and folded into the keep-mask multiplier).  End-to-end
coefficient error ~1e-6 relative to scale (fp32-envelope level) at 1 cyc/row
PE throughput instead of fp32's 4.

Entropy per node n uses the identity
    -sum_k q ln q  =  (ln(Se) * S - G) / Se,   q = sq/Se,
    S = sum_k c^2,  Se = S + 1e-8,  G = sum_k sq*ln(sq)
with ACT (Square with rho scale, Ln), DVE (segmented reduces, small vector
math), GPSIMD (sq*ln(sq) product) and ACT masking (Copy with per-node scale).

Sharding: pure data parallel, 16 batch rows per core across 8 NeuronCores.
"""
import sys

sys.path.insert(0, "/opt/trn_rl_repo")

import numpy as np
import ml_dtypes

import concourse.bass as bass
import concourse.tile as tile
from concourse import bacc, mybir
from concourse import bass_utils

F32 = mybir.dt.float32
BF16 = mybir.dt.bfloat16
ALU = mybir.AluOpType
ACTFN = mybir.ActivationFunctionType

N_CORES = 8
B = 128                  # total batch rows
T = 65536                # samples per row
ROWS = B // N_CORES      # rows per core (16)
LEVEL = 8
NODES = 1 << LEVEL       # 256
BLK = T // NODES         # 256 samples per block
THRESHOLD = 0.1
RPB = 2                  # rows per batch (inner tile loop)
NBATCH = ROWS // RPB     # 8 batches per core

_INV = np.float32(0.7071067811865476)
_g = np.arange(NODES)
_FREQ_PERM = np.argsort(_g ^ (_g >> 1))


def _build_w():
    """W'[n, j]: response of freq-ordered node n to impulse at in-block pos j."""
    c = np.eye(BLK, dtype=np.float32)[:, None, :]
    for _ in range(LEVEL):
        ev = c[..., 0::2]
        od = c[..., 1::2]
        a = (ev + od) * _INV
        d = (ev - od) * _INV
        c = np.concatenate([a[:, :, None, :], d[:, :, None, :]], axis=2)
        c = c.reshape(BLK, -1, a.shape[-1])
    w = c[:, _FREQ_PERM, 0].T.copy()  # [n, j]
    return w


W = _build_w()
_WMAG = float(np.abs(W[0, 0]))            # single magnitude of all entries
RHO = np.float32(_WMAG / 0.0625)          # exact uniform correction factor
# bf16 weight matrix: +-0.0625 exactly; lhsT chunks packed
# wt[j', jc*256 + nh*128 + m] = sign(W[nh*128+m, jc*128+j']) * 0.0625
_WT_SIGNED = np.sign(np.hstack([W.T[0:128, :], W.T[128:256, :]])) * 0.0625
WT_PACKED = _WT_SIGNED.astype(ml_dtypes.bfloat16).copy()
IDENT = np.eye(128, dtype=np.float32)

_MODULE_CACHE = None


def _build_module():
    nc = bacc.Bacc("TRN2", target_bir_lowering=False, debug=False,
                   enable_asserts=False, num_devices=N_CORES)
    x_d = nc.dram_tensor("x", [ROWS, T], F32, kind="ExternalInput").ap()
    wt_d = nc.dram_tensor("wt", [128, 512], BF16, kind="ExternalInput").ap()
    id_d = nc.dram_tensor("ident", [128, 128], F32, kind="ExternalInput").ap()
    out_d = nc.dram_tensor("out", [ROWS, T], F32, kind="ExternalOutput").ap()
    ent_d = nc.dram_tensor("ent", [ROWS, NODES], F32, kind="ExternalOutput").ap()

    FREE = RPB * 512     # free-dim elems per batch tile (2 rows x 512)

    with tile.TileContext(nc) as tc:
        with (
            tc.tile_pool(name="const", bufs=1) as const_pool,
            tc.tile_pool(name="xin", bufs=4) as xin_pool,
            tc.tile_pool(name="xt_ps", bufs=2, space="PSUM") as xtps_pool,
            tc.tile_pool(name="c_ps", bufs=3, space="PSUM") as cps_pool,
            tc.tile_pool(name="hi_sb", bufs=3) as hi_pool,
            tc.tile_pool(name="lo_sb", bufs=3) as lo_pool,
            tc.tile_pool(name="sq", bufs=3) as sq_pool,
            tc.tile_pool(name="lq", bufs=3) as lq_pool,
            tc.tile_pool(name="tt", bufs=3) as tt_pool,
            tc.tile_pool(name="outs", bufs=4) as out_pool,
            tc.tile_pool(name="stats", bufs=3) as stat_pool,
        ):
            wt_sb = const_pool.tile([128, 512], BF16)
            nc.sync.dma_start(wt_sb[:], wt_d)
            ident = const_pool.tile([128, 128], F32)
            nc.sync.dma_start(ident[:], id_d)
            ent_sb = const_pool.tile([128, 2 * ROWS], F32)
            bias_tiny = const_pool.tile([128, 1], F32)
            nc.gpsimd.memset(bias_tiny[:], 1e-30)

            for bi in range(NBATCH):
                r0 = bi * RPB
                x_sb = xin_pool.tile([128, FREE], F32, tag="x_sb")
                src = x_d[r0:r0 + RPB].rearrange(
                    "r (cj p j) -> p r cj j", cj=2, p=128, j=BLK)
                nc.sync.dma_start(x_sb[:], src)

                # PE transposes (exact fp32), per row rl into a 1-bank PSUM
                # tile with cols (jc, cj, k'); hi/lo split scattered jc-major:
                #   hi/lo cols = jc*512 + rl*256 + cj*128 + k'
                hi = hi_pool.tile([128, FREE], BF16, tag="hi")
                lo = lo_pool.tile([128, FREE], BF16, tag="lo")
                hi_v = hi[:].rearrange("p (jc rl cj kk) -> p jc rl cj kk",
                                       jc=2, rl=RPB, cj=2, kk=128)
                lo_v = lo[:].rearrange("p (jc rl cj kk) -> p jc rl cj kk",
                                       jc=2, rl=RPB, cj=2, kk=128)
                for rl in range(RPB):
                    xt_r = xtps_pool.tile([128, 512], F32, tag="xt")
                    for cj in range(2):
                        for jc in range(2):
                            nc.tensor.transpose(
                                xt_r[:, (jc * 2 + cj) * 128:
                                     (jc * 2 + cj + 1) * 128],
                                x_sb[:, rl * 512 + cj * 256 + jc * 128:
                                     rl * 512 + cj * 256 + (jc + 1) * 128],
                                ident[:],
                            )
                    # hi = bf16(xt) (PSUM->SBUF cast); alternate engines
                    if rl % 2 == 0:
                        nc.vector.tensor_copy(hi_v[:, :, rl], xt_r[:])
                    else:
                        nc.scalar.copy(hi_v[:, :, rl], xt_r[:])
                    # lo = bf16(xt - hi)  (DVE; fp32 internal math)
                    nc.vector.tensor_tensor(lo_v[:, :, rl], xt_r[:],
                                            hi_v[:, :, rl], op=ALU.subtract)

                # bf16 matmuls, N=512 contiguous moving operands, hi+lo
                # accumulated:  c[:, nh*512 + rl*256 + k] = c_raw[nh*128+n', k]
                c = cps_pool.tile([128, FREE], F32, tag="c")
                for nh in range(2):
                    mm = 0
                    for jc in range(2):
                        for part in (hi, lo):
                            nc.tensor.matmul(
                                c[:, nh * 512:(nh + 1) * 512],
                                wt_sb[:, jc * 256 + nh * 128:
                                      jc * 256 + (nh + 1) * 128],
                                part[:, jc * 512:(jc + 1) * 512],
                                start=(mm == 0), stop=(mm == 3),
                            )
                            mm += 1

                # entropy pipeline; sq = (rho * c_raw)^2 restores exact scale
                sq = sq_pool.tile([128, FREE], F32, tag="sq")
                nc.scalar.activation(sq[:], c[:], ACTFN.Square,
                                     scale=float(RHO))
                s4 = stat_pool.tile([128, 2 * RPB], F32, tag="s4")
                nc.vector.tensor_reduce(
                    s4[:], sq[:].rearrange("p (g k) -> p g k", k=BLK),
                    axis=mybir.AxisListType.X, op=ALU.add)
                lq = lq_pool.tile([128, FREE], F32, tag="lq")
                nc.scalar.activation(lq[:], sq[:], ACTFN.Ln, bias=bias_tiny[:])
                t = tt_pool.tile([128, FREE], F32, tag="t")
                nc.gpsimd.tensor_tensor(t[:], sq[:], lq[:], op=ALU.mult)
                g4 = stat_pool.tile([128, 2 * RPB], F32, tag="g4")
                nc.vector.tensor_reduce(
                    g4[:], t[:].rearrange("p (g k) -> p g k", k=BLK),
                    axis=mybir.AxisListType.X, op=ALU.add)

                # entropy smalls on [128, 4]
                se = stat_pool.tile([128, 2 * RPB], F32, tag="se")
                nc.vector.tensor_scalar_add(se[:], s4[:], 1e-8)
                sinv = stat_pool.tile([128, 2 * RPB], F32, tag="sinv")
                nc.vector.reciprocal(sinv[:], se[:])
                lns = stat_pool.tile([128, 2 * RPB], F32, tag="lns")
                nc.scalar.activation(lns[:], se[:], ACTFN.Ln)
                u = stat_pool.tile([128, 2 * RPB], F32, tag="u")
                nc.vector.tensor_tensor(u[:], s4[:], lns[:], op=ALU.mult)
                v = stat_pool.tile([128, 2 * RPB], F32, tag="v")
                nc.vector.tensor_tensor(v[:], u[:], g4[:], op=ALU.subtract)
                # e columns are (nh, rl); ent_sb wants col 2*(r0+rl)+nh
                e = ent_sb[:, 2 * r0: 2 * (r0 + RPB)].rearrange(
                    "p (rl nh) -> p nh rl", rl=RPB, nh=2)
                nc.vector.tensor_tensor(e, v[:], sinv[:], op=ALU.mult)
                # m4 = (entropy > thr ? 1 : 0) * rho  — rho folded into mask
                m4 = stat_pool.tile([128, 2 * RPB], F32, tag="m4")
                nc.vector.tensor_scalar(
                    m4[:], ent_sb[:, 2 * r0: 2 * (r0 + RPB)].rearrange(
                        "p (rl nh) -> p nh rl", rl=RPB, nh=2),
                    THRESHOLD, float(RHO), op0=ALU.is_gt, op1=ALU.mult)

                # masked output on ACT; out layout (rl, nh, k) for the DMA
                o = out_pool.tile([128, FREE], F32, tag="o")
                for rl in range(RPB):
                    for nh in range(2):
                        csl = c[:, nh * 512 + rl * 256: nh * 512 + (rl + 1) * 256]
                        osl = o[:, rl * 512 + nh * 256: rl * 512 + (nh + 1) * 256]
                        msl = m4[:, nh * RPB + rl: nh * RPB + rl + 1]
                        nc.scalar.activation(osl, csl, ACTFN.Copy, scale=msl)
                dst = out_d[r0:r0 + RPB].rearrange(
                    "r (nh p k) -> p r nh k", nh=2, p=128, k=BLK)
                nc.sync.dma_start(dst, o[:])

            # entropy epilogue: [128 n', 32 (r,nh)] -> transpose -> [32, 128] -> DRAM
            entT_ps = xtps_pool.tile([128, 512], F32, tag="xt")
            nc.tensor.transpose(entT_ps[0:2 * ROWS, 0:128], ent_sb[:], ident[:])
            entT = stat_pool.tile([2 * ROWS, 128], F32, tag="entT_sb")
            nc.vector.tensor_copy(entT[:], entT_ps[0:2 * ROWS, 0:128])
            nc.sync.dma_start(
                ent_d.rearrange("r (nh n) -> (r nh) n", nh=2), entT[:])

    nc.compile()
    return nc


def _get_module():
    global _MODULE_CACHE
    if _MODULE_CACHE is None:
        _MODULE_CACHE = _build_module()
    return _MODULE_CACHE


def kernel(x: np.ndarray) -> tuple[np.ndarray, np.ndarray, np.ndarray]:
    x = np.ascontiguousarray(np.asarray(x, dtype=np.float32))
    assert x.shape == (B, T)
    nc = _get_module()
    in_maps = []
    for core in range(N_CORES):
        shard = x[core * ROWS:(core + 1) * ROWS]
        in_maps.append({"x": shard, "wt": WT_PACKED, "ident": IDENT})
    res = bass_utils.run_bass_kernel_spmd(nc, in_maps, core_ids=list(range(N_CORES)))
    global LAST_RESULTS
    LAST_RESULTS = res
    out = np.empty((B, NODES, BLK), dtype=np.float32)
    ent = np.empty((B, NODES), dtype=np.float32)
    for core in range(N_CORES):
        r = res.results[core]
        out[core * ROWS:(core + 1) * ROWS] = r["out"].reshape(ROWS, NODES, BLK)
        ent[core * ROWS:(core + 1) * ROWS] = r["ent"]
    keep = ent > np.float32(THRESHOLD)
    return out, ent, keep


# revision 25
# speedup vs baseline: 1.0738x; 1.0738x over previous
"""Trainium2 Bass kernel for DWPEDecomposition.

Problem: x [128, 65536] f32.  For each batch row: full 8-level Haar (db1)
wavelet-packet tree -> [256 nodes, 256 coeffs] in frequency order, per-node
Shannon entropy of the normalized squared coefficients, and a keep mask
(entropy > 0.1) that zeroes pruned nodes' coefficients.

Key insight: the 8-level Haar packet cascade on a 65536-sample row is exactly
a 256x256 Walsh-Hadamard-style matrix W' (entries +-(1/sqrt2)^8, with the
frequency-order node permutation folded in) applied independently to each of
the 256 contiguous 256-sample blocks:

    coeffs[b, n, k] = sum_j W'[n, j] * x[b, 256*k + j]

which maps onto the TensorEngine as (PE-transpose + matmul) per tile.

Matmul precision scheme ("bf16_hilo"): all W' entries share one magnitude, so
bf16(W') = +-0.0625 = W'/rho exactly, with rho a single uniform scalar.  The
transposed x is split into bf16 hi + bf16 lo (x ~ hi+lo to ~2^-17); the four
accumulating bf16 matmuls per output bank compute c_raw = bf16(W') @ (hi+lo)
in fp32 PSUM, and the uniform rho is re-applied exactly downstream (Square's
scale immediate, and folded into the keep-mask multiplier).  End-to-end
coefficient error ~1e-6 relative to scale (fp32-envelope level) at 1 cyc/row
PE throughput instead of fp32's 4.

Entropy per node n uses the identity
    -sum_k q ln q  =  (ln(Se) * S - G) / Se,   q = sq/Se,
    S = sum_k c^2,  Se = S + 1e-8,  G = sum_k sq*ln(sq)
with ACT (Square with rho scale, Ln), DVE (segmented reduces, small vector
math), GPSIMD (sq*ln(sq) product) and ACT masking (Copy with per-node scale).

Sharding: pure data parallel, 16 batch rows per core across 8 NeuronCores.
"""
import sys

sys.path.insert(0, "/opt/trn_rl_repo")

import numpy as np
import ml_dtypes

import concourse.bass as bass
import concourse.tile as tile
from concourse import bacc, mybir
from concourse import bass_utils

F32 = mybir.dt.float32
BF16 = mybir.dt.bfloat16
ALU = mybir.AluOpType
ACTFN = mybir.ActivationFunctionType

N_CORES = 8
B = 128                  # total batch rows
T = 65536                # samples per row
ROWS = B // N_CORES      # rows per core (16)
LEVEL = 8
NODES = 1 << LEVEL       # 256
BLK = T // NODES         # 256 samples per block
THRESHOLD = 0.1
RPB = 2                  # rows per batch (inner tile loop)
NBATCH = ROWS // RPB     # 8 batches per core

_INV = np.float32(0.7071067811865476)
_g = np.arange(NODES)
_FREQ_PERM = np.argsort(_g ^ (_g >> 1))


def _build_w():
    """W'[n, j]: response of freq-ordered node n to impulse at in-block pos j."""
    c = np.eye(BLK, dtype=np.float32)[:, None, :]
    for _ in range(LEVEL):
        ev = c[..., 0::2]
        od = c[..., 1::2]
        a = (ev + od) * _INV
        d = (ev - od) * _INV
        c = np.concatenate([a[:, :, None, :], d[:, :, None, :]], axis=2)
        c = c.reshape(BLK, -1, a.shape[-1])
    w = c[:, _FREQ_PERM, 0].T.copy()  # [n, j]
    return w


W = _build_w()
_WMAG = float(np.abs(W[0, 0]))            # single magnitude of all entries
RHO = np.float32(_WMAG / 0.0625)          # exact uniform correction factor
# bf16 weight matrix: +-0.0625 exactly; lhsT chunks packed
# wt[j', jc*256 + nh*128 + m] = sign(W[nh*128+m, jc*128+j']) * 0.0625
_WT_SIGNED = np.sign(np.hstack([W.T[0:128, :], W.T[128:256, :]])) * 0.0625
WT_PACKED = _WT_SIGNED.astype(ml_dtypes.bfloat16).copy()
IDENT = np.eye(128, dtype=np.float32)

_MODULE_CACHE = None


def _build_module():
    nc = bacc.Bacc("TRN2", target_bir_lowering=False, debug=False,
                   enable_asserts=False, num_devices=N_CORES)
    x_d = nc.dram_tensor("x", [ROWS, T], F32, kind="ExternalInput").ap()
    wt_d = nc.dram_tensor("wt", [128, 512], BF16, kind="ExternalInput").ap()
    id_d = nc.dram_tensor("ident", [128, 128], F32, kind="ExternalInput").ap()
    out_d = nc.dram_tensor("out", [ROWS, T], F32, kind="ExternalOutput").ap()
    ent_d = nc.dram_tensor("ent", [ROWS, NODES], F32, kind="ExternalOutput").ap()

    FREE = RPB * 512     # free-dim elems per batch tile (2 rows x 512)

    with tile.TileContext(nc) as tc:
        with (
            tc.tile_pool(name="const", bufs=1) as const_pool,
            tc.tile_pool(name="xin", bufs=4) as xin_pool,
            tc.tile_pool(name="xt_ps", bufs=2, space="PSUM") as xtps_pool,
            tc.tile_pool(name="c_ps", bufs=3, space="PSUM") as cps_pool,
            tc.tile_pool(name="hi_sb", bufs=3) as hi_pool,
            tc.tile_pool(name="lo_sb", bufs=3) as lo_pool,
            tc.tile_pool(name="sq", bufs=4) as sq_pool,
            tc.tile_pool(name="lq", bufs=4) as lq_pool,
            tc.tile_pool(name="tt", bufs=4) as tt_pool,
            tc.tile_pool(name="outs", bufs=5) as out_pool,
            tc.tile_pool(name="stats", bufs=8) as stat_pool,
        ):
            wt_sb = const_pool.tile([128, 512], BF16)
            nc.sync.dma_start(wt_sb[:], wt_d)
            ident = const_pool.tile([128, 128], F32)
            nc.sync.dma_start(ident[:], id_d)
            ent_sb = const_pool.tile([128, 2 * ROWS], F32)
            bias_tiny = const_pool.tile([128, 1], F32)
            nc.gpsimd.memset(bias_tiny[:], 1e-30)

            for bi in range(NBATCH):
                r0 = bi * RPB
                x_sb = xin_pool.tile([128, FREE], F32, tag="x_sb")
                src = x_d[r0:r0 + RPB].rearrange(
                    "r (cj p j) -> p r cj j", cj=2, p=128, j=BLK)
                nc.sync.dma_start(x_sb[:], src)

                # PE transposes (exact fp32), per row rl into a 1-bank PSUM
                # tile with cols (jc, cj, k'); hi/lo split scattered jc-major:
                #   hi/lo cols = jc*512 + rl*256 + cj*128 + k'
                hi = hi_pool.tile([128, FREE], BF16, tag="hi")
                lo = lo_pool.tile([128, FREE], BF16, tag="lo")
                hi_v = hi[:].rearrange("p (jc rl cj kk) -> p jc rl cj kk",
                                       jc=2, rl=RPB, cj=2, kk=128)
                lo_v = lo[:].rearrange("p (jc rl cj kk) -> p jc rl cj kk",
                                       jc=2, rl=RPB, cj=2, kk=128)
                for rl in range(RPB):
                    xt_r = xtps_pool.tile([128, 512], F32, tag="xt")
                    for cj in range(2):
                        for jc in range(2):
                            nc.tensor.transpose(
                                xt_r[:, (jc * 2 + cj) * 128:
                                     (jc * 2 + cj + 1) * 128],
                                x_sb[:, rl * 512 + cj * 256 + jc * 128:
                                     rl * 512 + cj * 256 + (jc + 1) * 128],
                                ident[:],
                            )
                    # hi = bf16(xt) (PSUM->SBUF cast); alternate engines
                    if rl % 2 == 0:
                        nc.vector.tensor_copy(hi_v[:, :, rl], xt_r[:])
                    else:
                        nc.scalar.copy(hi_v[:, :, rl], xt_r[:])
                    # lo = bf16(xt - hi)  (DVE; fp32 internal math)
                    nc.vector.tensor_tensor(lo_v[:, :, rl], xt_r[:],
                                            hi_v[:, :, rl], op=ALU.subtract)

                # bf16 matmuls, N=512 contiguous moving operands, hi+lo
                # accumulated:  c[:, nh*512 + rl*256 + k] = c_raw[nh*128+n', k]
                c = cps_pool.tile([128, FREE], F32, tag="c")
                for nh in range(2):
                    mm = 0
                    for part in (hi, lo):     # hi first: lo prod off crit path
                        for jc in range(2):
                            nc.tensor.matmul(
                                c[:, nh * 512:(nh + 1) * 512],
                                wt_sb[:, jc * 256 + nh * 128:
                                      jc * 256 + (nh + 1) * 128],
                                part[:, jc * 512:(jc + 1) * 512],
                                start=(mm == 0), stop=(mm == 3),
                            )
                            mm += 1

                # entropy pipeline; sq = (rho * c_raw)^2 restores exact scale
                sq = sq_pool.tile([128, FREE], F32, tag="sq")
                nc.scalar.activation(sq[:], c[:], ACTFN.Square,
                                     scale=float(RHO))
                s4 = stat_pool.tile([128, 2 * RPB], F32, tag="s4")
                nc.vector.tensor_reduce(
                    s4[:], sq[:].rearrange("p (g k) -> p g k", k=BLK),
                    axis=mybir.AxisListType.X, op=ALU.add)
                lq = lq_pool.tile([128, FREE], F32, tag="lq")
                nc.scalar.activation(lq[:], sq[:], ACTFN.Ln, bias=bias_tiny[:])
                t = tt_pool.tile([128, FREE], F32, tag="t")
                g4 = stat_pool.tile([128, 2 * RPB], F32, tag="g4")
                H = FREE // 2
                for half in range(2):
                    nc.gpsimd.tensor_tensor(
                        t[:, half * H:(half + 1) * H],
                        sq[:, half * H:(half + 1) * H],
                        lq[:, half * H:(half + 1) * H], op=ALU.mult)
                    nc.vector.tensor_reduce(
                        g4[:, half * 2:(half + 1) * 2],
                        t[:, half * H:(half + 1) * H].rearrange(
                            "p (g k) -> p g k", k=BLK),
                        axis=mybir.AxisListType.X, op=ALU.add)

                # entropy smalls on [128, 4]
                se = stat_pool.tile([128, 2 * RPB], F32, tag="se")
                nc.vector.tensor_scalar_add(se[:], s4[:], 1e-8)
                sinv = stat_pool.tile([128, 2 * RPB], F32, tag="sinv")
                nc.vector.reciprocal(sinv[:], se[:])
                lns = stat_pool.tile([128, 2 * RPB], F32, tag="lns")
                nc.scalar.activation(lns[:], se[:], ACTFN.Ln)
                u = stat_pool.tile([128, 2 * RPB], F32, tag="u")
                nc.vector.tensor_tensor(u[:], s4[:], lns[:], op=ALU.mult)
                v = stat_pool.tile([128, 2 * RPB], F32, tag="v")
                nc.vector.tensor_tensor(v[:], u[:], g4[:], op=ALU.subtract)
                # e columns are (nh, rl); ent_sb wants col 2*(r0+rl)+nh
                e = ent_sb[:, 2 * r0: 2 * (r0 + RPB)].rearrange(
                    "p (rl nh) -> p nh rl", rl=RPB, nh=2)
                nc.vector.tensor_tensor(e, v[:], sinv[:], op=ALU.mult)
                # m4 = (entropy > thr ? 1 : 0) * rho  — rho folded into mask
                m4 = stat_pool.tile([128, 2 * RPB], F32, tag="m4")
                nc.vector.tensor_scalar(
                    m4[:], ent_sb[:, 2 * r0: 2 * (r0 + RPB)].rearrange(
                        "p (rl nh) -> p nh rl", rl=RPB, nh=2),
                    THRESHOLD, float(RHO), op0=ALU.is_gt, op1=ALU.mult)

                # masked output on ACT; out layout (rl, nh, k) for the DMA
                o = out_pool.tile([128, FREE], F32, tag="o")
                for rl in range(RPB):
                    for nh in range(2):
                        csl = c[:, nh * 512 + rl * 256: nh * 512 + (rl + 1) * 256]
                        osl = o[:, rl * 512 + nh * 256: rl * 512 + (nh + 1) * 256]
                        msl = m4[:, nh * RPB + rl: nh * RPB + rl + 1]
                        nc.scalar.activation(osl, csl, ACTFN.Copy, scale=msl)
                dst = out_d[r0:r0 + RPB].rearrange(
                    "r (nh p k) -> p r nh k", nh=2, p=128, k=BLK)
                nc.sync.dma_start(dst, o[:])

            # entropy epilogue: [128 n', 32 (r,nh)] -> transpose -> [32, 128] -> DRAM
            entT_ps = xtps_pool.tile([128, 512], F32, tag="xt")
            nc.tensor.transpose(entT_ps[0:2 * ROWS, 0:128], ent_sb[:], ident[:])
            entT = stat_pool.tile([2 * ROWS, 128], F32, tag="entT_sb")
            nc.vector.tensor_copy(entT[:], entT_ps[0:2 * ROWS, 0:128])
            nc.sync.dma_start(
                ent_d.rearrange("r (nh n) -> (r nh) n", nh=2), entT[:])

    nc.compile()
    return nc


def _get_module():
    global _MODULE_CACHE
    if _MODULE_CACHE is None:
        _MODULE_CACHE = _build_module()
    return _MODULE_CACHE


def kernel(x: np.ndarray) -> tuple[np.ndarray, np.ndarray, np.ndarray]:
    x = np.ascontiguousarray(np.asarray(x, dtype=np.float32))
    assert x.shape == (B, T)
    nc = _get_module()
    in_maps = []
    for core in range(N_CORES):
        shard = x[core * ROWS:(core + 1) * ROWS]
        in_maps.append({"x": shard, "wt": WT_PACKED, "ident": IDENT})
    res = bass_utils.run_bass_kernel_spmd(nc, in_maps, core_ids=list(range(N_CORES)))
    global LAST_RESULTS
    LAST_RESULTS = res
    out = np.empty((B, NODES, BLK), dtype=np.float32)
    ent = np.empty((B, NODES), dtype=np.float32)
    for core in range(N_CORES):
        r = res.results[core]
        out[core * ROWS:(core + 1) * ROWS] = r["out"].reshape(ROWS, NODES, BLK)
        ent[core * ROWS:(core + 1) * ROWS] = r["ent"]
    keep = ent > np.float32(THRESHOLD)
    return out, ent, keep


# revision 27
# speedup vs baseline: 1.1076x; 1.0315x over previous
"""Trainium2 Bass kernel for DWPEDecomposition.

Problem: x [128, 65536] f32.  For each batch row: full 8-level Haar (db1)
wavelet-packet tree -> [256 nodes, 256 coeffs] in frequency order, per-node
Shannon entropy of the normalized squared coefficients, and a keep mask
(entropy > 0.1) that zeroes pruned nodes' coefficients.

Key insight: the 8-level Haar packet cascade on a 65536-sample row is exactly
a 256x256 Walsh-Hadamard-style matrix W' (entries +-(1/sqrt2)^8, with the
frequency-order node permutation folded in) applied independently to each of
the 256 contiguous 256-sample blocks:

    coeffs[b, n, k] = sum_j W'[n, j] * x[b, 256*k + j]

which maps onto the TensorEngine as (PE-transpose + matmul) per tile.

Matmul precision scheme ("bf16_hilo"): all W' entries share one magnitude, so
bf16(W') = +-0.0625 = W'/rho exactly, with rho a single uniform scalar.  The
transposed x is split into bf16 hi + bf16 lo (x ~ hi+lo to ~2^-17); the four
accumulating bf16 matmuls per output bank compute c_raw = bf16(W') @ (hi+lo)
in fp32 PSUM, and the uniform rho is re-applied exactly downstream (Square's
scale immediate, and folded into the keep-mask multiplier).  End-to-end
coefficient error ~1e-6 relative to scale (fp32-envelope level) at 1 cyc/row
PE throughput instead of fp32's 4.

Entropy per node n uses the identity
    -sum_k q ln q  =  (ln(Se) * S - G) / Se,   q = sq/Se,
    S = sum_k c^2,  Se = S + 1e-8,  G = sum_k sq*ln(sq)
with ACT (Square with rho scale, Ln), DVE (segmented reduces, small vector
math), GPSIMD (sq*ln(sq) product) and ACT masking (Copy with per-node scale).

Sharding: pure data parallel, 16 batch rows per core across 8 NeuronCores.
"""
import sys

sys.path.insert(0, "/opt/trn_rl_repo")

import numpy as np
import ml_dtypes

import concourse.bass as bass
import concourse.tile as tile
from concourse import bacc, mybir
from concourse import bass_utils

F32 = mybir.dt.float32
BF16 = mybir.dt.bfloat16
ALU = mybir.AluOpType
ACTFN = mybir.ActivationFunctionType

N_CORES = 8
B = 128                  # total batch rows
T = 65536                # samples per row
ROWS = B // N_CORES      # rows per core (16)
LEVEL = 8
NODES = 1 << LEVEL       # 256
BLK = T // NODES         # 256 samples per block
THRESHOLD = 0.1
RPB = 2                  # rows per batch (inner tile loop)
NBATCH = ROWS // RPB     # 8 batches per core

_INV = np.float32(0.7071067811865476)
_g = np.arange(NODES)
_FREQ_PERM = np.argsort(_g ^ (_g >> 1))


def _build_w():
    """W'[n, j]: response of freq-ordered node n to impulse at in-block pos j."""
    c = np.eye(BLK, dtype=np.float32)[:, None, :]
    for _ in range(LEVEL):
        ev = c[..., 0::2]
        od = c[..., 1::2]
        a = (ev + od) * _INV
        d = (ev - od) * _INV
        c = np.concatenate([a[:, :, None, :], d[:, :, None, :]], axis=2)
        c = c.reshape(BLK, -1, a.shape[-1])
    w = c[:, _FREQ_PERM, 0].T.copy()  # [n, j]
    return w


W = _build_w()
_WMAG = float(np.abs(W[0, 0]))            # single magnitude of all entries
RHO = np.float32(_WMAG / 0.0625)          # exact uniform correction factor
# bf16 weight matrix: +-0.0625 exactly; lhsT chunks packed
# wt[j', jc*256 + nh*128 + m] = sign(W[nh*128+m, jc*128+j']) * 0.0625
_WT_SIGNED = np.sign(np.hstack([W.T[0:128, :], W.T[128:256, :]])) * 0.0625
WT_PACKED = _WT_SIGNED.astype(ml_dtypes.bfloat16).copy()
IDENT = np.eye(128, dtype=np.float32)

_MODULE_CACHE = None


def _build_module():
    nc = bacc.Bacc("TRN2", target_bir_lowering=False, debug=False,
                   enable_asserts=False, num_devices=N_CORES)
    x_d = nc.dram_tensor("x", [ROWS, T], F32, kind="ExternalInput").ap()
    wt_d = nc.dram_tensor("wt", [128, 512], BF16, kind="ExternalInput").ap()
    id_d = nc.dram_tensor("ident", [128, 128], F32, kind="ExternalInput").ap()
    out_d = nc.dram_tensor("out", [ROWS, T], F32, kind="ExternalOutput").ap()
    ent_d = nc.dram_tensor("ent", [ROWS, NODES], F32, kind="ExternalOutput").ap()

    FREE = RPB * 512     # free-dim elems per batch tile (2 rows x 512)

    with tile.TileContext(nc) as tc:
        with (
            tc.tile_pool(name="const", bufs=1) as const_pool,
            tc.tile_pool(name="xin", bufs=6) as xin_pool,
            tc.tile_pool(name="xt_ps", bufs=2, space="PSUM") as xtps_pool,
            tc.tile_pool(name="c_ps", bufs=3, space="PSUM") as cps_pool,
            tc.tile_pool(name="hi_sb", bufs=4) as hi_pool,
            tc.tile_pool(name="lo_sb", bufs=4) as lo_pool,
            tc.tile_pool(name="sq", bufs=4) as sq_pool,
            tc.tile_pool(name="lq", bufs=4) as lq_pool,
            tc.tile_pool(name="tt", bufs=4) as tt_pool,
            tc.tile_pool(name="outs", bufs=6) as out_pool,
            tc.tile_pool(name="stats", bufs=8) as stat_pool,
        ):
            wt_sb = const_pool.tile([128, 512], BF16)
            nc.sync.dma_start(wt_sb[:], wt_d)
            ident = const_pool.tile([128, 128], F32)
            nc.sync.dma_start(ident[:], id_d)
            ent_sb = const_pool.tile([128, 2 * ROWS], F32)
            bias_tiny = const_pool.tile([128, 1], F32)
            nc.gpsimd.memset(bias_tiny[:], 1e-30)

            for bi in range(NBATCH):
                r0 = bi * RPB
                x_sb = xin_pool.tile([128, FREE], F32, tag="x_sb")
                src = x_d[r0:r0 + RPB].rearrange(
                    "r (cj p j) -> p r cj j", cj=2, p=128, j=BLK)
                nc.sync.dma_start(x_sb[:], src)

                # PE transposes (exact fp32), per row rl into a 1-bank PSUM
                # tile with cols (jc, cj, k'); hi/lo split scattered jc-major:
                #   hi/lo cols = jc*512 + rl*256 + cj*128 + k'
                hi = hi_pool.tile([128, FREE], BF16, tag="hi")
                lo = lo_pool.tile([128, FREE], BF16, tag="lo")
                hi_v = hi[:].rearrange("p (jc rl cj kk) -> p jc rl cj kk",
                                       jc=2, rl=RPB, cj=2, kk=128)
                lo_v = lo[:].rearrange("p (jc rl cj kk) -> p jc rl cj kk",
                                       jc=2, rl=RPB, cj=2, kk=128)
                for rl in range(RPB):
                    xt_r = xtps_pool.tile([128, 512], F32, tag="xt")
                    for cj in range(2):
                        for jc in range(2):
                            nc.tensor.transpose(
                                xt_r[:, (jc * 2 + cj) * 128:
                                     (jc * 2 + cj + 1) * 128],
                                x_sb[:, rl * 512 + cj * 256 + jc * 128:
                                     rl * 512 + cj * 256 + (jc + 1) * 128],
                                ident[:],
                            )
                    # hi = bf16(xt) (PSUM->SBUF cast); alternate engines
                    if rl % 2 == 0:
                        nc.vector.tensor_copy(hi_v[:, :, rl], xt_r[:])
                    else:
                        nc.scalar.copy(hi_v[:, :, rl], xt_r[:])
                    # lo = bf16(xt - hi)  (DVE; fp32 internal math)
                    nc.vector.tensor_tensor(lo_v[:, :, rl], xt_r[:],
                                            hi_v[:, :, rl], op=ALU.subtract)

                # bf16 matmuls, N=512 contiguous moving operands, hi+lo
                # accumulated:  c[:, nh*512 + rl*256 + k] = c_raw[nh*128+n', k]
                c = cps_pool.tile([128, FREE], F32, tag="c")
                for nh in range(2):
                    mm = 0
                    for part in (hi, lo):     # hi first: lo prod off crit path
                        for jc in range(2):
                            nc.tensor.matmul(
                                c[:, nh * 512:(nh + 1) * 512],
                                wt_sb[:, jc * 256 + nh * 128:
                                      jc * 256 + (nh + 1) * 128],
                                part[:, jc * 512:(jc + 1) * 512],
                                start=(mm == 0), stop=(mm == 3),
                            )
                            mm += 1

                # entropy pipeline; sq = (rho * c_raw)^2 restores exact scale
                sq = sq_pool.tile([128, FREE], F32, tag="sq")
                nc.scalar.activation(sq[:], c[:], ACTFN.Square,
                                     scale=float(RHO))
                s4 = stat_pool.tile([128, 2 * RPB], F32, tag="s4")
                nc.vector.tensor_reduce(
                    s4[:], sq[:].rearrange("p (g k) -> p g k", k=BLK),
                    axis=mybir.AxisListType.X, op=ALU.add)
                # S-only smalls hoisted here: off the post-G critical path
                se = stat_pool.tile([128, 2 * RPB], F32, tag="se")
                nc.vector.tensor_scalar_add(se[:], s4[:], 1e-8)
                sinv = stat_pool.tile([128, 2 * RPB], F32, tag="sinv")
                nc.vector.reciprocal(sinv[:], se[:])
                lns = stat_pool.tile([128, 2 * RPB], F32, tag="lns")
                nc.scalar.activation(lns[:], se[:], ACTFN.Ln)
                u = stat_pool.tile([128, 2 * RPB], F32, tag="u")
                nc.vector.tensor_tensor(u[:], s4[:], lns[:], op=ALU.mult)

                lq = lq_pool.tile([128, FREE], F32, tag="lq")
                nc.scalar.activation(lq[:], sq[:], ACTFN.Ln, bias=bias_tiny[:])
                t = tt_pool.tile([128, FREE], F32, tag="t")
                g4 = stat_pool.tile([128, 2 * RPB], F32, tag="g4")
                H = FREE // 2
                for half in range(2):
                    nc.gpsimd.tensor_tensor(
                        t[:, half * H:(half + 1) * H],
                        sq[:, half * H:(half + 1) * H],
                        lq[:, half * H:(half + 1) * H], op=ALU.mult)
                    nc.vector.tensor_reduce(
                        g4[:, half * 2:(half + 1) * 2],
                        t[:, half * H:(half + 1) * H].rearrange(
                            "p (g k) -> p g k", k=BLK),
                        axis=mybir.AxisListType.X, op=ALU.add)

                # post-G smalls: 3 same-engine DVE hops only
                v = stat_pool.tile([128, 2 * RPB], F32, tag="v")
                nc.vector.tensor_tensor(v[:], u[:], g4[:], op=ALU.subtract)
                # e columns are (nh, rl); ent_sb wants col 2*(r0+rl)+nh
                e = ent_sb[:, 2 * r0: 2 * (r0 + RPB)].rearrange(
                    "p (rl nh) -> p nh rl", rl=RPB, nh=2)
                nc.vector.tensor_tensor(e, v[:], sinv[:], op=ALU.mult)
                # m4 = (entropy > thr ? 1 : 0) * rho  — rho folded into mask
                m4 = stat_pool.tile([128, 2 * RPB], F32, tag="m4")
                nc.vector.tensor_scalar(
                    m4[:], ent_sb[:, 2 * r0: 2 * (r0 + RPB)].rearrange(
                        "p (rl nh) -> p nh rl", rl=RPB, nh=2),
                    THRESHOLD, float(RHO), op0=ALU.is_gt, op1=ALU.mult)

                # masked output on ACT; out layout (rl, nh, k) for the DMA
                o = out_pool.tile([128, FREE], F32, tag="o")
                for rl in range(RPB):
                    for nh in range(2):
                        csl = c[:, nh * 512 + rl * 256: nh * 512 + (rl + 1) * 256]
                        osl = o[:, rl * 512 + nh * 256: rl * 512 + (nh + 1) * 256]
                        msl = m4[:, nh * RPB + rl: nh * RPB + rl + 1]
                        nc.scalar.activation(osl, csl, ACTFN.Copy, scale=msl)
                dst = out_d[r0:r0 + RPB].rearrange(
                    "r (nh p k) -> p r nh k", nh=2, p=128, k=BLK)
                nc.sync.dma_start(dst, o[:])

            # entropy epilogue: [128 n', 32 (r,nh)] -> transpose -> [32, 128] -> DRAM
            entT_ps = xtps_pool.tile([128, 512], F32, tag="xt")
            nc.tensor.transpose(entT_ps[0:2 * ROWS, 0:128], ent_sb[:], ident[:])
            entT = stat_pool.tile([2 * ROWS, 128], F32, tag="entT_sb")
            nc.vector.tensor_copy(entT[:], entT_ps[0:2 * ROWS, 0:128])
            nc.sync.dma_start(
                ent_d.rearrange("r (nh n) -> (r nh) n", nh=2), entT[:])

    nc.compile()
    return nc


def _get_module():
    global _MODULE_CACHE
    if _MODULE_CACHE is None:
        _MODULE_CACHE = _build_module()
    return _MODULE_CACHE


def kernel(x: np.ndarray) -> tuple[np.ndarray, np.ndarray, np.ndarray]:
    x = np.ascontiguousarray(np.asarray(x, dtype=np.float32))
    assert x.shape == (B, T)
    nc = _get_module()
    in_maps = []
    for core in range(N_CORES):
        shard = x[core * ROWS:(core + 1) * ROWS]
        in_maps.append({"x": shard, "wt": WT_PACKED, "ident": IDENT})
    res = bass_utils.run_bass_kernel_spmd(nc, in_maps, core_ids=list(range(N_CORES)))
    global LAST_RESULTS
    LAST_RESULTS = res
    out = np.empty((B, NODES, BLK), dtype=np.float32)
    ent = np.empty((B, NODES), dtype=np.float32)
    for core in range(N_CORES):
        r = res.results[core]
        out[core * ROWS:(core + 1) * ROWS] = r["out"].reshape(ROWS, NODES, BLK)
        ent[core * ROWS:(core + 1) * ROWS] = r["ent"]
    keep = ent > np.float32(THRESHOLD)
    return out, ent, keep
